# revision 2
# baseline (speedup 1.0000x reference)
"""Trainium2 Bass kernel for LocalizationLoss (box MSE + cross-entropy, batch mean).

Input : output [262144, 1004] f32  (cols 0:4 = box pred cx,cy,w,h; cols 4:1004 = logits)
        target [262144, 5]    f32  (xmin,ymin,xmax,ymax,class_id)
Output: scalar f32 = mean_b( mean_4((box_pred-box_true)^2) + CE(logits, class) )

Strategy (pure data parallel over 8 cores, 32768 rows each):
  - rows mapped p-major: partition p owns rows p*256..p*256+255 of its shard
  - stream 16 groups of 16 row-tiles [128, 16, 1004]; one big DMA per group
  - ScalarE: exp over logits with fused row-sum (accum_out) -> sumexp per row
  - VectorE: picked logit via one scalar_tensor_tensor per tile:
        out = (iota is_equal class_p) * logits ; accum_out = sum = logits[p, class_p]
    (iota is a [128,1000] constant input 0..999 per row; class_p is the f32
     class id as a per-partition scalar AP)
  - box-loss math batched per group on [128,16] strided views; squared-error
    sums via ScalarE Square activation with accum_out into per-group slots
  - epilogue: logZ = Ln(sumexp) with fused sum; CE_sum = logZ_sum - picked_sum
  - each core returns [128,1] per-partition partial sums; host adds and /B

This container's walrus build accepts at most ONE sync-wait per instruction,
while the Tile scheduler attaches several. `_split_multiwait_bir` rewrites the
serialized BIR to hoist extra waits onto single-wait NoOp carriers, and is
installed as a wrapper around compile_bir_kernel at import time.
"""

import json as _json

import numpy as np

import concourse.bass as bass
import concourse.tile as tile
from concourse import mybir
import concourse.bass_utils as _bass_utils
import concourse.bass2jax as _bass2jax
from concourse.bass_utils import run_bass_kernel_spmd

P = 128
B = 262144
C = 1004
NCLS = 1000
NCORES = 8
R = B // NCORES       # 32768 rows per core
T = R // P            # 256 row-tiles per core (rows per partition)
G = 16                # row-tiles per group
NG = T // G           # 16 groups

F32 = mybir.dt.float32
ALU = mybir.AluOpType
ACTF = mybir.ActivationFunctionType


# --------------------------------------------------------------------------
# BIR post-pass: this image's walrus supports only one sync-wait per
# instruction; split extras onto NoOp carriers placed just before.
# --------------------------------------------------------------------------
def _split_multiwait_bir(bir_json: bytes) -> bytes:
    d = _json.loads(bir_json)
    changed = False
    for fn in d.get("functions", []):
        for blk in fn.get("blocks", []):
            insts = blk.get("instructions", [])
            out = []
            for ins in insts:
                si = ins.get("sync_info") or {}
                waits = si.get("on_wait") or []
                if len(waits) > 1:
                    changed = True
                    for i, w in enumerate(waits[:-1]):
                        out.append(
                            {
                                "debug": ins.get("debug", 0),
                                "engine": ins["engine"],
                                "ins": [],
                                "name": f"{ins['name']}-wsplit{i}",
                                "opcode": "NoOp",
                                "outs": [],
                                "sync_info": {"on_update": [], "on_wait": [w]},
                            }
                        )
                    ins["sync_info"]["on_wait"] = [waits[-1]]
                out.append(ins)
            blk["instructions"] = out
    if not changed:
        return bir_json
    return _json.dumps(d).encode()


_orig_compile_bir_kernel = _bass_utils.compile_bir_kernel


def _compile_bir_kernel_fixed(bir_json, tmpdir, neff_name="file.neff"):
    if isinstance(bir_json, str):
        bir_json = bir_json.encode()
    return _orig_compile_bir_kernel(_split_multiwait_bir(bir_json), tmpdir, neff_name)


if _bass_utils.compile_bir_kernel is not _compile_bir_kernel_fixed:
    _bass_utils.compile_bir_kernel = _compile_bir_kernel_fixed
    _bass2jax.compile_bir_kernel = _compile_bir_kernel_fixed


# --------------------------------------------------------------------------
# kernel build
# --------------------------------------------------------------------------
def build():
    nc = bass.Bass()
    x = nc.dram_tensor("x", [R, C], F32, kind="ExternalInput")
    t = nc.dram_tensor("t", [R, 5], F32, kind="ExternalInput")
    iota_in = nc.dram_tensor("iota", [P, NCLS], F32, kind="ExternalInput")
    out = nc.dram_tensor("partial", [P, 1], F32, kind="ExternalOutput")

    xv = x[:].rearrange("(p n) c -> p n c", p=P)   # [128, 256, 1004]
    tv = t[:].rearrange("(p n) f -> p n f", p=P)   # [128, 256, 5]

    with tile.TileContext(nc) as tc:
        with (
            tc.tile_pool(name="data", bufs=2) as data_pool,
            tc.tile_pool(name="tgt", bufs=2) as tgt_pool,
            tc.tile_pool(name="scr", bufs=2) as scr_pool,
            tc.tile_pool(name="acc", bufs=1) as acc_pool,
        ):
            iota_t = acc_pool.tile([P, NCLS], F32)
            nc.sync.dma_start(out=iota_t, in_=iota_in[:])

            sumexp_all = acc_pool.tile([P, T], F32)      # per-row sum(exp(logits))
            picked_all = acc_pool.tile([P, T], F32)      # per-row logits[class]
            loc_all = acc_pool.tile([P, NG * 4], F32)    # per-(group,comp) sq-err sums

            for grp in range(NG):
                t0 = grp * G
                data = data_pool.tile([P, G, C], F32, tag="data")
                tgt = tgt_pool.tile([P, G, 5], F32, tag="tgt")
                nc.sync.dma_start(out=data, in_=xv[:, t0 : t0 + G, :])
                nc.sync.dma_start(out=tgt, in_=tv[:, t0 : t0 + G, :])

                u = scr_pool.tile([P, G], F32, tag="u")
                e = scr_pool.tile([P, G], F32, tag="e")
                sq = scr_pool.tile([P, G], F32, tag="sq")
                # cx, cy: err = 0.5*(t_lo + t_hi) - pred  (sign irrelevant, squared)
                for i, j in ((0, 0), (1, 1)):
                    nc.vector.tensor_add(u, tgt[:, :, i], tgt[:, :, i + 2])
                    nc.vector.scalar_tensor_tensor(
                        e, u, 0.5, data[:, :, j], ALU.mult, ALU.subtract
                    )
                    nc.scalar.activation(
                        out=sq, in_=e, func=ACTF.Square,
                        accum_out=loc_all[:, grp * 4 + j : grp * 4 + j + 1],
                    )
                # w, h: err = (t_hi - t_lo) - pred
                for i, j in ((0, 2), (1, 3)):
                    nc.vector.tensor_sub(u, tgt[:, :, i + 2], tgt[:, :, i])
                    nc.vector.tensor_sub(e, u, data[:, :, j])
                    nc.scalar.activation(
                        out=sq, in_=e, func=ACTF.Square,
                        accum_out=loc_all[:, grp * 4 + j : grp * 4 + j + 1],
                    )

                for g in range(G):
                    tt = t0 + g
                    exp_scr = scr_pool.tile([P, NCLS], F32, tag="exp_scr")
                    nc.scalar.activation(
                        out=exp_scr,
                        in_=data[:, g, 4:C],
                        func=ACTF.Exp,
                        accum_out=sumexp_all[:, tt : tt + 1],
                    )
                    pick_scr = scr_pool.tile([P, NCLS], F32, tag="pick_scr")
                    nc.vector.scalar_tensor_tensor(
                        pick_scr,
                        iota_t,
                        tgt[:, g, 4:5],
                        data[:, g, 4:C],
                        ALU.is_equal,
                        ALU.mult,
                        accum_out=picked_all[:, tt : tt + 1],
                    )

            # ---- epilogue ----
            logz_scr = acc_pool.tile([P, T], F32)
            logz_sum = acc_pool.tile([P, 1], F32)
            nc.scalar.activation(
                out=logz_scr, in_=sumexp_all, func=ACTF.Ln, accum_out=logz_sum
            )
            pick_sum = acc_pool.tile([P, 1], F32)
            nc.vector.tensor_reduce(
                out=pick_sum, in_=picked_all, axis=mybir.AxisListType.X, op=ALU.add
            )
            loc_sum = acc_pool.tile([P, 1], F32)
            nc.vector.tensor_reduce(
                out=loc_sum, in_=loc_all, axis=mybir.AxisListType.X, op=ALU.add
            )
            s = acc_pool.tile([P, 1], F32)
            # s = 0.25*loc_sum + logz_sum - pick_sum
            nc.vector.scalar_tensor_tensor(
                s, loc_sum, 0.25, logz_sum, ALU.mult, ALU.add
            )
            nc.vector.tensor_sub(s, s, pick_sum)
            nc.sync.dma_start(out=out[:], in_=s)

    return nc


_IOTA = np.ascontiguousarray(
    np.broadcast_to(np.arange(NCLS, dtype=np.float32), (P, NCLS))
)


def _run(output, target, **spmd_kwargs):
    output = np.ascontiguousarray(np.asarray(output, dtype=np.float32))
    target = np.ascontiguousarray(np.asarray(target, dtype=np.float32))
    assert output.shape == (B, C), output.shape
    assert target.shape == (B, 5), target.shape
    nc = build()
    in_maps = [
        {
            "x": output[i * R : (i + 1) * R],
            "t": target[i * R : (i + 1) * R],
            "iota": _IOTA,
        }
        for i in range(NCORES)
    ]
    res = run_bass_kernel_spmd(nc, in_maps, core_ids=list(range(NCORES)), **spmd_kwargs)
    total = 0.0
    for r in res.results:
        total += r["partial"].astype(np.float64).sum()
    return np.float32(total / B), res


def kernel(output, target):
    val, _ = _run(output, target)
    return np.asarray(val, dtype=np.float32)


def kernel_profiled(output, target, **kw):
    """Returns (scalar, BassKernelResults) with trace for perf analysis."""
    return _run(output, target, trace=True, **kw)
